# revision 3
# baseline (speedup 1.0000x reference)
"""GCN bond layer on 8 NeuronCores (Trainium2, Bass/Tile).

Distribution: edge-parallel. Each core gets E/8 edges; the host shards
edge_attr and the per-edge endpoint feature rows (ghost/halo replication
of x along the edge shard) plus the tiny weights. On device, each core
runs the three 16x16 linears as one 128x128 block-diagonal matmul over a
feature-major layout (8 edge groups x 16 features on partitions),
accumulates BatchNorm partial sums, all-reduces them across the 8 cores,
and applies normalize+ReLU+residual in a second streaming pass.

The Linear biases b1+b2+b3 shift every edge equally, so they cancel
inside BatchNorm: var is shift-invariant and the mean shift absorbs
them. The device therefore never adds biases; it computes
  a = gamma * rsqrt(var(e_nb) + eps),  c = beta - mean(e_nb) * a
and out = edge_attr + relu(e_nb * a + c), which equals the reference.
"""

import sys

if "/opt/trn_rl_repo" not in sys.path:
    sys.path.insert(0, "/opt/trn_rl_repo")

import numpy as np
from contextlib import ExitStack

import concourse.bass as bass
import concourse.tile as tile
from concourse import bacc, mybir
from concourse.bass_utils import run_bass_kernel_spmd

N_CORES = 8
EMBD = 16
GROUPS = 8
BN_EPS = 1e-5

_build_cache = {}


def _build(ec_padded, tile_cols, n_edges_total):
    """Build + compile the Bass program for one edge-shard size.

    ec_padded: padded edges per core (= n_tiles * GROUPS * tile_cols)
    tile_cols: free-dim columns per tile (each column holds GROUPS edges)
    n_edges_total: real edge count across all cores (BN denominator)
    """
    key = (ec_padded, tile_cols, n_edges_total)
    if key in _build_cache:
        return _build_cache[key]

    edges_per_tile = GROUPS * tile_cols
    assert ec_padded % edges_per_tile == 0
    nt = ec_padded // edges_per_tile
    cols = nt * tile_cols
    f32 = mybir.dt.float32

    nc = bacc.Bacc("TRN2", target_bir_lowering=False, debug=False,
                   num_devices=N_CORES)

    ea_d = nc.dram_tensor("ea", [128, cols], f32, kind="ExternalInput").ap()
    hs_d = nc.dram_tensor("hs", [128, cols], f32, kind="ExternalInput").ap()
    hd_d = nc.dram_tensor("hd", [128, cols], f32, kind="ExternalInput").ap()
    w1_d = nc.dram_tensor("w1bd", [128, 128], f32, kind="ExternalInput").ap()
    w2_d = nc.dram_tensor("w2bd", [128, 128], f32, kind="ExternalInput").ap()
    w3_d = nc.dram_tensor("w3bd", [128, 128], f32, kind="ExternalInput").ap()
    fold_d = nc.dram_tensor("foldm", [128, EMBD], f32, kind="ExternalInput").ap()
    bc_d = nc.dram_tensor("bcm", [EMBD, 128], f32, kind="ExternalInput").ap()
    gb_d = nc.dram_tensor("gb", [EMBD, 2], f32, kind="ExternalInput").ap()
    out_d = nc.dram_tensor("out", [128, cols], f32, kind="ExternalOutput").ap()

    MM = 512  # free-dim per matmul (one PSUM bank span)
    assert tile_cols % MM == 0
    ctx = ExitStack()
    with tile.TileContext(nc) as tc, ctx:
        consts = ctx.enter_context(tc.tile_pool(name="consts", bufs=1))
        streams = ctx.enter_context(tc.tile_pool(name="streams", bufs=3))
        work = ctx.enter_context(tc.tile_pool(name="work", bufs=2))
        stats = ctx.enter_context(tc.tile_pool(name="stats", bufs=1))
        psum = ctx.enter_context(tc.tile_pool(name="psum", bufs=2, space="PSUM"))
        dram = ctx.enter_context(tc.tile_pool(name="dram", bufs=2, space="DRAM"))

        w1_sb = consts.tile([128, 128], f32, tag="w1")
        nc.sync.dma_start(w1_sb[:], w1_d[:])
        w2_sb = consts.tile([128, 128], f32, tag="w2")
        nc.sync.dma_start(w2_sb[:], w2_d[:])
        w3_sb = consts.tile([128, 128], f32, tag="w3")
        nc.sync.dma_start(w3_sb[:], w3_d[:])
        fold_sb = consts.tile([128, EMBD], f32, tag="fold")
        nc.sync.dma_start(fold_sb[:], fold_d[:])
        bc_sb = consts.tile([EMBD, 128], f32, tag="bc")
        nc.sync.dma_start(bc_sb[:], bc_d[:])
        gb_sb = consts.tile([EMBD, 2], f32, tag="gb")
        nc.sync.dma_start(gb_sb[:], gb_d[:])

        s1cols = stats.tile([128, nt], f32, tag="s1")
        s2cols = stats.tile([128, nt], f32, tag="s2")

        def load_and_matmul(t):
            ea_t = streams.tile([128, tile_cols], f32, tag="ea")
            nc.sync.dma_start(ea_t[:], ea_d[:, bass.ts(t, tile_cols)])
            hs_t = streams.tile([128, tile_cols], f32, tag="hs")
            nc.sync.dma_start(hs_t[:], hs_d[:, bass.ts(t, tile_cols)])
            hd_t = streams.tile([128, tile_cols], f32, tag="hd")
            nc.sync.dma_start(hd_t[:], hd_d[:, bass.ts(t, tile_cols)])
            pe_t = psum.tile([128, tile_cols], f32, tag="pe")
            for m in range(tile_cols // MM):
                sl = bass.ts(m, MM)
                nc.tensor.matmul(pe_t[:, sl], w1_sb[:], ea_t[:, sl],
                                 start=True, stop=False)
                nc.tensor.matmul(pe_t[:, sl], w2_sb[:], hs_t[:, sl],
                                 start=False, stop=False)
                nc.tensor.matmul(pe_t[:, sl], w3_sb[:], hd_t[:, sl],
                                 start=False, stop=True)
            return ea_t, pe_t

        # ---- pass 1: accumulate BN partial sums -------------------------
        for t in range(nt):
            _, pe_t = load_and_matmul(t)
            cp = work.tile([128, tile_cols], f32, tag="cp")
            nc.scalar.activation(cp[:], pe_t[:],
                                 mybir.ActivationFunctionType.Copy,
                                 accum_out=s1cols[:, t:t + 1])
            sq = work.tile([128, tile_cols], f32, tag="sq")
            nc.scalar.activation(sq[:], pe_t[:],
                                 mybir.ActivationFunctionType.Square,
                                 accum_out=s2cols[:, t:t + 1])

        # ---- stats: reduce, all-reduce, scale/shift ---------------------
        st_sb = stats.tile([128, 2], f32, tag="st")
        nc.vector.tensor_reduce(st_sb[:, 0:1], s1cols[:],
                                mybir.AxisListType.X, mybir.AluOpType.add)
        nc.vector.tensor_reduce(st_sb[:, 1:2], s2cols[:],
                                mybir.AxisListType.X, mybir.AluOpType.add)
        cc_in = dram.tile([128, 2], f32, tag="ccin")
        cc_out = dram.tile([128, 2], f32, tag="ccout")
        nc.sync.dma_start(cc_in[:], st_sb[:])
        nc.gpsimd.collective_compute(
            "AllReduce", mybir.AluOpType.add,
            replica_groups=[list(range(N_CORES))],
            ins=[cc_in.opt()], outs=[cc_out.opt()])
        st_ar = stats.tile([128, 2], f32, tag="star")
        nc.sync.dma_start(st_ar[:], cc_out[:])

        p16 = psum.tile([EMBD, 2], f32, tag="pe")
        nc.tensor.matmul(p16[:], fold_sb[:], st_ar[:], start=True, stop=True)
        inv_e = 1.0 / float(n_edges_total)
        m16 = stats.tile([EMBD, 1], f32, tag="m16")
        nc.scalar.activation(m16[:], p16[:, 0:1],
                             mybir.ActivationFunctionType.Copy, scale=inv_e)
        q16 = stats.tile([EMBD, 1], f32, tag="q16")
        nc.scalar.activation(q16[:], p16[:, 1:2],
                             mybir.ActivationFunctionType.Copy, scale=inv_e)
        msq = stats.tile([EMBD, 1], f32, tag="msq")
        nc.scalar.square(msq[:], m16[:])
        var = stats.tile([EMBD, 1], f32, tag="var")
        nc.vector.tensor_sub(var[:], q16[:], msq[:])
        nc.vector.tensor_scalar_add(var[:], var[:], BN_EPS)
        # ACT Sqrt is table-based (~1e-3 rel); one Newton step fixes it:
        # s1 = 0.5*(s0 + v/s0), then rs = 1/s1 via the exact DVE reciprocal.
        s0 = stats.tile([EMBD, 1], f32, tag="sd")
        nc.scalar.sqrt(s0[:], var[:])
        r0 = stats.tile([EMBD, 1], f32, tag="r0")
        nc.vector.reciprocal(r0[:], s0[:])
        vr = stats.tile([EMBD, 1], f32, tag="vr")
        nc.vector.tensor_mul(vr[:], var[:], r0[:])
        s1 = stats.tile([EMBD, 1], f32, tag="s1n")
        nc.vector.tensor_add(s1[:], s0[:], vr[:])
        nc.vector.tensor_scalar_mul(s1[:], s1[:], 0.5)
        rs = stats.tile([EMBD, 1], f32, tag="rs")
        nc.vector.reciprocal(rs[:], s1[:])
        ac16 = stats.tile([EMBD, 2], f32, tag="ac16")
        nc.vector.tensor_mul(ac16[:, 0:1], rs[:], gb_sb[:, 0:1])
        t16 = stats.tile([EMBD, 1], f32, tag="t16")
        nc.vector.tensor_mul(t16[:], m16[:], ac16[:, 0:1])
        nc.vector.tensor_sub(ac16[:, 1:2], gb_sb[:, 1:2], t16[:])
        pac = psum.tile([128, 2], f32, tag="pe")
        nc.tensor.matmul(pac[:], bc_sb[:], ac16[:], start=True, stop=True)
        ac_bc = stats.tile([128, 2], f32, tag="acbc")
        nc.scalar.copy(ac_bc[:], pac[:])

        # ---- pass 2: recompute, normalize, relu, residual ---------------
        for t in range(nt):
            ea_t, pe_t = load_and_matmul(t)
            r_sb = work.tile([128, tile_cols], f32, tag="cp")
            nc.scalar.activation(r_sb[:], pe_t[:],
                                 mybir.ActivationFunctionType.Relu,
                                 bias=ac_bc[:, 1:2], scale=ac_bc[:, 0:1])
            o_sb = work.tile([128, tile_cols], f32, tag="sq")
            nc.vector.tensor_add(o_sb[:], r_sb[:], ea_t[:])
            nc.sync.dma_start(out_d[:, bass.ts(t, tile_cols)], o_sb[:])

    nc.compile()
    _build_cache[key] = (nc, nt, cols)
    return _build_cache[key]


def _to_t8(arr, nt, tile_cols, ec_padded):
    """[ec_padded, 16] edge-major -> [128, nt*tile_cols] feature-major
    8-group layout: out[16g+f, t*tile_cols+j] = arr[t*G*C + g*C + j, f]."""
    a = arr.reshape(nt, GROUPS, tile_cols, EMBD)
    return np.ascontiguousarray(
        a.transpose(1, 3, 0, 2).reshape(128, nt * tile_cols))


def _from_t8(mat, nt, tile_cols, ec_padded):
    a = mat.reshape(GROUPS, EMBD, nt, tile_cols)
    return np.ascontiguousarray(
        a.transpose(2, 0, 3, 1).reshape(ec_padded, EMBD))


def kernel(x, edge_index, edge_attr, w1, b1, w2, b2, w3, b3, gamma, beta):
    x = np.asarray(x, dtype=np.float32)
    edge_index = np.asarray(edge_index)
    edge_attr = np.asarray(edge_attr, dtype=np.float32)
    w1 = np.asarray(w1, dtype=np.float32)
    w2 = np.asarray(w2, dtype=np.float32)
    w3 = np.asarray(w3, dtype=np.float32)
    gamma = np.asarray(gamma, dtype=np.float32)
    beta = np.asarray(beta, dtype=np.float32)

    n_edges = edge_attr.shape[0]
    assert n_edges % N_CORES == 0
    ec = n_edges // N_CORES
    tile_cols = 2048 if ec >= 16384 else 512
    ept = GROUPS * tile_cols
    nt = -(-ec // ept)
    ec_padded = nt * ept

    nc, nt_b, cols = _build(ec_padded, tile_cols, n_edges)

    eye = np.eye(GROUPS, dtype=np.float32)
    w1bd = np.kron(eye, w1.T.astype(np.float32)).astype(np.float32)
    w2bd = np.kron(eye, w2.T.astype(np.float32)).astype(np.float32)
    w3bd = np.kron(eye, w3.T.astype(np.float32)).astype(np.float32)
    foldm = np.tile(np.eye(EMBD, dtype=np.float32), (GROUPS, 1))
    bcm = np.tile(np.eye(EMBD, dtype=np.float32), (1, GROUPS))
    gb = np.stack([gamma, beta], axis=1).astype(np.float32)

    src = edge_index[0].astype(np.int64)
    dst = edge_index[1].astype(np.int64)

    in_maps = []
    for c in range(N_CORES):
        sl = slice(c * ec, (c + 1) * ec)
        ea_p = np.zeros((ec_padded, EMBD), np.float32)
        ea_p[:ec] = edge_attr[sl]
        hs_p = np.zeros((ec_padded, EMBD), np.float32)
        hs_p[:ec] = x[src[sl]]
        hd_p = np.zeros((ec_padded, EMBD), np.float32)
        hd_p[:ec] = x[dst[sl]]
        in_maps.append({
            "ea": _to_t8(ea_p, nt, tile_cols, ec_padded),
            "hs": _to_t8(hs_p, nt, tile_cols, ec_padded),
            "hd": _to_t8(hd_p, nt, tile_cols, ec_padded),
            "w1bd": w1bd, "w2bd": w2bd, "w3bd": w3bd,
            "foldm": foldm, "bcm": bcm, "gb": gb,
        })

    res = run_bass_kernel_spmd(nc, in_maps, list(range(N_CORES)))
    global last_exec_ns
    last_exec_ns = res.exec_time_ns

    out = np.empty((n_edges, EMBD), np.float32)
    for c in range(N_CORES):
        rows = _from_t8(np.asarray(res.results[c]["out"]), nt, tile_cols,
                        ec_padded)
        out[c * ec:(c + 1) * ec] = rows[:ec]
    return out


# revision 9
# speedup vs baseline: 1.1149x; 1.1149x over previous
"""GCN bond layer on 8 NeuronCores (Trainium2, Bass/Tile).

Distribution: edge-parallel. Each core gets E/8 edges; the host shards
edge_attr and the per-edge endpoint feature rows (ghost/halo replication
of x along the edge shard) plus the tiny weights. On device, each core
runs the three 16x16 linears as one 128x128 block-diagonal matmul over a
feature-major layout (8 edge groups x 16 features on partitions),
accumulates BatchNorm partial sums, all-reduces them across the 8 cores,
and applies normalize+ReLU+residual in a second streaming pass.

The Linear biases b1+b2+b3 shift every edge equally, so they cancel
inside BatchNorm: var is shift-invariant and the mean shift absorbs
them. The device therefore never adds biases; it computes
  a = gamma * rsqrt(var(e_nb) + eps),  c = beta - mean(e_nb) * a
and out = edge_attr + relu(e_nb * a + c), which equals the reference.
"""

import sys

if "/opt/trn_rl_repo" not in sys.path:
    sys.path.insert(0, "/opt/trn_rl_repo")

import numpy as np
from contextlib import ExitStack

import concourse.bass as bass
import concourse.tile as tile
from concourse import bacc, mybir
from concourse.bass_utils import run_bass_kernel_spmd

N_CORES = 8
EMBD = 16
GROUPS = 8
BN_EPS = 1e-5

_build_cache = {}


def _build(ec_padded, tile_cols, n_edges_total):
    """Build + compile the Bass program for one edge-shard size.

    ec_padded: padded edges per core (= n_tiles * GROUPS * tile_cols)
    tile_cols: free-dim columns per tile (each column holds GROUPS edges)
    n_edges_total: real edge count across all cores (BN denominator)
    """
    key = (ec_padded, tile_cols, n_edges_total)
    if key in _build_cache:
        return _build_cache[key]

    edges_per_tile = GROUPS * tile_cols
    assert ec_padded % edges_per_tile == 0
    nt = ec_padded // edges_per_tile
    cols = nt * tile_cols
    f32 = mybir.dt.float32

    nc = bacc.Bacc("TRN2", target_bir_lowering=False, debug=False,
                   num_devices=N_CORES)

    ea_d = nc.dram_tensor("ea", [128, cols], f32, kind="ExternalInput").ap()
    hs_d = nc.dram_tensor("hs", [128, cols], f32, kind="ExternalInput").ap()
    hd_d = nc.dram_tensor("hd", [128, cols], f32, kind="ExternalInput").ap()
    w1_d = nc.dram_tensor("w1bd", [128, 128], f32, kind="ExternalInput").ap()
    w2_d = nc.dram_tensor("w2bd", [128, 128], f32, kind="ExternalInput").ap()
    w3_d = nc.dram_tensor("w3bd", [128, 128], f32, kind="ExternalInput").ap()
    fold_d = nc.dram_tensor("foldm", [128, EMBD], f32, kind="ExternalInput").ap()
    bc_d = nc.dram_tensor("bcm", [EMBD, 128], f32, kind="ExternalInput").ap()
    gb_d = nc.dram_tensor("gb", [EMBD, 2], f32, kind="ExternalInput").ap()
    out_d = nc.dram_tensor("out", [128, cols], f32, kind="ExternalOutput").ap()
    ecache_d = nc.dram_tensor("ecache", [128, cols], f32).ap()

    MM = 512  # free-dim per matmul (one PSUM bank span)
    assert tile_cols % MM == 0
    ctx = ExitStack()
    with tile.TileContext(nc) as tc, ctx:
        consts = ctx.enter_context(tc.tile_pool(name="consts", bufs=1))
        streams = ctx.enter_context(tc.tile_pool(name="streams", bufs=4))
        work = ctx.enter_context(tc.tile_pool(name="work", bufs=2))
        stats = ctx.enter_context(tc.tile_pool(name="stats", bufs=1))
        psum = ctx.enter_context(tc.tile_pool(name="psum", bufs=2, space="PSUM"))
        dram = ctx.enter_context(tc.tile_pool(name="dram", bufs=2, space="DRAM"))

        w1_sb = consts.tile([128, 128], f32, tag="w1")
        nc.sync.dma_start(w1_sb[:], w1_d[:])
        w2_sb = consts.tile([128, 128], f32, tag="w2")
        nc.sync.dma_start(w2_sb[:], w2_d[:])
        w3_sb = consts.tile([128, 128], f32, tag="w3")
        nc.sync.dma_start(w3_sb[:], w3_d[:])
        fold_sb = consts.tile([128, EMBD], f32, tag="fold")
        nc.sync.dma_start(fold_sb[:], fold_d[:])
        bc_sb = consts.tile([EMBD, 128], f32, tag="bc")
        nc.sync.dma_start(bc_sb[:], bc_d[:])
        gb_sb = consts.tile([EMBD, 2], f32, tag="gb")
        nc.sync.dma_start(gb_sb[:], gb_d[:])

        s1cols = stats.tile([128, nt], f32, tag="s1")
        s2cols = stats.tile([128, nt], f32, tag="s2")

        def load_and_matmul(t):
            ea_t = streams.tile([128, tile_cols], f32, tag="ea")
            nc.sync.dma_start(ea_t[:], ea_d[:, bass.ts(t, tile_cols)])
            hs_t = streams.tile([128, tile_cols], f32, tag="hs")
            nc.sync.dma_start(hs_t[:], hs_d[:, bass.ts(t, tile_cols)])
            hd_t = streams.tile([128, tile_cols], f32, tag="hd")
            nc.sync.dma_start(hd_t[:], hd_d[:, bass.ts(t, tile_cols)])
            pe_t = psum.tile([128, tile_cols], f32, tag="pe")
            for m in range(tile_cols // MM):
                sl = bass.ts(m, MM)
                nc.tensor.matmul(pe_t[:, sl], w1_sb[:], ea_t[:, sl],
                                 start=True, stop=False)
                nc.tensor.matmul(pe_t[:, sl], w2_sb[:], hs_t[:, sl],
                                 start=False, stop=False)
                nc.tensor.matmul(pe_t[:, sl], w3_sb[:], hd_t[:, sl],
                                 start=False, stop=True)
            return ea_t, pe_t

        # ---- pass 1: accumulate BN partial sums, cache e_nb -------------
        # Sum of squares on DVE (exact multiply) — the ACT Square table has
        # a ~0.5% systematic bias which corrupts the variance.
        for t in range(nt):
            _, pe_t = load_and_matmul(t)
            cp = work.tile([128, tile_cols], f32, tag="cp")
            nc.scalar.activation(cp[:], pe_t[:],
                                 mybir.ActivationFunctionType.Copy,
                                 accum_out=s1cols[:, t:t + 1])
            sq = work.tile([128, tile_cols], f32, tag="sq")
            nc.vector.tensor_mul(sq[:], cp[:], cp[:])
            nc.vector.tensor_reduce(s2cols[:, t:t + 1], sq[:],
                                    mybir.AxisListType.X, mybir.AluOpType.add)
            nc.sync.dma_start(ecache_d[:, bass.ts(t, tile_cols)], cp[:])

        # ---- stats: reduce, all-reduce, scale/shift ---------------------
        st_sb = stats.tile([128, 2], f32, tag="st")
        nc.vector.tensor_reduce(st_sb[:, 0:1], s1cols[:],
                                mybir.AxisListType.X, mybir.AluOpType.add)
        nc.vector.tensor_reduce(st_sb[:, 1:2], s2cols[:],
                                mybir.AxisListType.X, mybir.AluOpType.add)
        cc_in = dram.tile([128, 2], f32, tag="ccin")
        cc_out = dram.tile([128, 2], f32, tag="ccout")
        nc.sync.dma_start(cc_in[:], st_sb[:])
        nc.gpsimd.collective_compute(
            "AllReduce", mybir.AluOpType.add,
            replica_groups=[list(range(N_CORES))],
            ins=[cc_in.opt()], outs=[cc_out.opt()])
        st_ar = stats.tile([128, 2], f32, tag="star")
        nc.sync.dma_start(st_ar[:], cc_out[:])

        p16 = psum.tile([EMBD, 2], f32, tag="pe")
        nc.tensor.matmul(p16[:], fold_sb[:], st_ar[:], start=True, stop=True)
        inv_e = 1.0 / float(n_edges_total)
        m16 = stats.tile([EMBD, 1], f32, tag="m16")
        nc.scalar.activation(m16[:], p16[:, 0:1],
                             mybir.ActivationFunctionType.Copy, scale=inv_e)
        q16 = stats.tile([EMBD, 1], f32, tag="q16")
        nc.scalar.activation(q16[:], p16[:, 1:2],
                             mybir.ActivationFunctionType.Copy, scale=inv_e)
        msq = stats.tile([EMBD, 1], f32, tag="msq")
        nc.scalar.square(msq[:], m16[:])
        var = stats.tile([EMBD, 1], f32, tag="var")
        nc.vector.tensor_sub(var[:], q16[:], msq[:])
        nc.vector.tensor_scalar_add(var[:], var[:], BN_EPS)
        # ACT Sqrt is table-based (~1e-3 rel); one Newton step fixes it:
        # s1 = 0.5*(s0 + v/s0), then rs = 1/s1 via the exact DVE reciprocal.
        s0 = stats.tile([EMBD, 1], f32, tag="sd")
        nc.scalar.sqrt(s0[:], var[:])
        r0 = stats.tile([EMBD, 1], f32, tag="r0")
        nc.vector.reciprocal(r0[:], s0[:])
        vr = stats.tile([EMBD, 1], f32, tag="vr")
        nc.vector.tensor_mul(vr[:], var[:], r0[:])
        s1 = stats.tile([EMBD, 1], f32, tag="s1n")
        nc.vector.tensor_add(s1[:], s0[:], vr[:])
        nc.vector.tensor_scalar_mul(s1[:], s1[:], 0.5)
        rs = stats.tile([EMBD, 1], f32, tag="rs")
        nc.vector.reciprocal(rs[:], s1[:])
        ac16 = stats.tile([EMBD, 2], f32, tag="ac16")
        nc.vector.tensor_mul(ac16[:, 0:1], rs[:], gb_sb[:, 0:1])
        t16 = stats.tile([EMBD, 1], f32, tag="t16")
        nc.vector.tensor_mul(t16[:], m16[:], ac16[:, 0:1])
        nc.vector.tensor_sub(ac16[:, 1:2], gb_sb[:, 1:2], t16[:])
        pac = psum.tile([128, 2], f32, tag="pe")
        nc.tensor.matmul(pac[:], bc_sb[:], ac16[:], start=True, stop=True)
        ac_bc = stats.tile([128, 2], f32, tag="acbc")
        nc.scalar.copy(ac_bc[:], pac[:])

        # ---- pass 2: read cached e_nb, normalize, relu, residual --------
        for t in range(nt):
            e_t = streams.tile([128, tile_cols], f32, tag="hs")
            nc.sync.dma_start(e_t[:], ecache_d[:, bass.ts(t, tile_cols)])
            ea_t = streams.tile([128, tile_cols], f32, tag="ea")
            nc.sync.dma_start(ea_t[:], ea_d[:, bass.ts(t, tile_cols)])
            r_sb = work.tile([128, tile_cols], f32, tag="cp")
            nc.scalar.activation(r_sb[:], e_t[:],
                                 mybir.ActivationFunctionType.Relu,
                                 bias=ac_bc[:, 1:2], scale=ac_bc[:, 0:1])
            o_sb = work.tile([128, tile_cols], f32, tag="sq")
            nc.vector.tensor_add(o_sb[:], r_sb[:], ea_t[:])
            nc.sync.dma_start(out_d[:, bass.ts(t, tile_cols)], o_sb[:])

    nc.compile()
    _build_cache[key] = (nc, nt, cols)
    return _build_cache[key]


def _to_t8(arr, nt, tile_cols, ec_padded):
    """[ec_padded, 16] edge-major -> [128, nt*tile_cols] feature-major
    8-group layout: out[16g+f, t*tile_cols+j] = arr[t*G*C + g*C + j, f]."""
    a = arr.reshape(nt, GROUPS, tile_cols, EMBD)
    return np.ascontiguousarray(
        a.transpose(1, 3, 0, 2).reshape(128, nt * tile_cols))


def _from_t8(mat, nt, tile_cols, ec_padded):
    a = mat.reshape(GROUPS, EMBD, nt, tile_cols)
    return np.ascontiguousarray(
        a.transpose(2, 0, 3, 1).reshape(ec_padded, EMBD))


def kernel(x, edge_index, edge_attr, w1, b1, w2, b2, w3, b3, gamma, beta):
    x = np.asarray(x, dtype=np.float32)
    edge_index = np.asarray(edge_index)
    edge_attr = np.asarray(edge_attr, dtype=np.float32)
    w1 = np.asarray(w1, dtype=np.float32)
    w2 = np.asarray(w2, dtype=np.float32)
    w3 = np.asarray(w3, dtype=np.float32)
    gamma = np.asarray(gamma, dtype=np.float32)
    beta = np.asarray(beta, dtype=np.float32)

    n_edges = edge_attr.shape[0]
    assert n_edges % N_CORES == 0
    ec = n_edges // N_CORES
    tile_cols = 2048 if ec >= 16384 else 512
    ept = GROUPS * tile_cols
    nt = -(-ec // ept)
    ec_padded = nt * ept

    nc, nt_b, cols = _build(ec_padded, tile_cols, n_edges)

    eye = np.eye(GROUPS, dtype=np.float32)
    w1bd = np.kron(eye, w1.T.astype(np.float32)).astype(np.float32)
    w2bd = np.kron(eye, w2.T.astype(np.float32)).astype(np.float32)
    w3bd = np.kron(eye, w3.T.astype(np.float32)).astype(np.float32)
    foldm = np.tile(np.eye(EMBD, dtype=np.float32), (GROUPS, 1))
    bcm = np.tile(np.eye(EMBD, dtype=np.float32), (1, GROUPS))
    gb = np.stack([gamma, beta], axis=1).astype(np.float32)

    src = edge_index[0].astype(np.int64)
    dst = edge_index[1].astype(np.int64)

    in_maps = []
    for c in range(N_CORES):
        sl = slice(c * ec, (c + 1) * ec)
        ea_p = np.zeros((ec_padded, EMBD), np.float32)
        ea_p[:ec] = edge_attr[sl]
        hs_p = np.zeros((ec_padded, EMBD), np.float32)
        hs_p[:ec] = x[src[sl]]
        hd_p = np.zeros((ec_padded, EMBD), np.float32)
        hd_p[:ec] = x[dst[sl]]
        in_maps.append({
            "ea": _to_t8(ea_p, nt, tile_cols, ec_padded),
            "hs": _to_t8(hs_p, nt, tile_cols, ec_padded),
            "hd": _to_t8(hd_p, nt, tile_cols, ec_padded),
            "w1bd": w1bd, "w2bd": w2bd, "w3bd": w3bd,
            "foldm": foldm, "bcm": bcm, "gb": gb,
        })

    res = run_bass_kernel_spmd(nc, in_maps, list(range(N_CORES)))
    global last_exec_ns
    last_exec_ns = res.exec_time_ns

    out = np.empty((n_edges, EMBD), np.float32)
    for c in range(N_CORES):
        rows = _from_t8(np.asarray(res.results[c]["out"]), nt, tile_cols,
                        ec_padded)
        out[c * ec:(c + 1) * ec] = rows[:ec]
    return out


# revision 10
# speedup vs baseline: 11.7906x; 10.5750x over previous
"""GCN bond layer on 8 NeuronCores (Trainium2, Bass/Tile).

Distribution: edge-parallel. Each core gets E/8 edges; the host shards
edge_attr and the per-edge endpoint feature rows (ghost/halo replication
of x along the edge shard) plus the tiny weights. On device, each core
runs the three 16x16 linears as one 128x128 block-diagonal matmul over a
feature-major layout (8 edge groups x 16 features on partitions),
accumulates BatchNorm partial sums, all-reduces them across the 8 cores,
and applies normalize+ReLU+residual in a second streaming pass.

The Linear biases b1+b2+b3 shift every edge equally, so they cancel
inside BatchNorm: var is shift-invariant and the mean shift absorbs
them. The device therefore never adds biases; it computes
  a = gamma * rsqrt(var(e_nb) + eps),  c = beta - mean(e_nb) * a
and out = edge_attr + relu(e_nb * a + c), which equals the reference.
"""

import sys

if "/opt/trn_rl_repo" not in sys.path:
    sys.path.insert(0, "/opt/trn_rl_repo")

import numpy as np
from contextlib import ExitStack

import concourse.bass as bass
import concourse.tile as tile
from concourse import bacc, mybir
from concourse.bass_utils import run_bass_kernel_spmd

N_CORES = 8
EMBD = 16
GROUPS = 8
BN_EPS = 1e-5

_build_cache = {}


def _build(ec_padded, tile_cols, n_edges_total):
    """Build + compile the Bass program for one edge-shard size.

    ec_padded: padded edges per core (= n_tiles * GROUPS * tile_cols)
    tile_cols: free-dim columns per tile (each column holds GROUPS edges)
    n_edges_total: real edge count across all cores (BN denominator)
    """
    key = (ec_padded, tile_cols, n_edges_total)
    if key in _build_cache:
        return _build_cache[key]

    edges_per_tile = GROUPS * tile_cols
    assert ec_padded % edges_per_tile == 0
    nt = ec_padded // edges_per_tile
    cols = nt * tile_cols
    f32 = mybir.dt.float32

    nc = bacc.Bacc("TRN2", target_bir_lowering=False, debug=False,
                   num_devices=N_CORES)

    ea_d = nc.dram_tensor("ea", [128, cols], f32, kind="ExternalInput").ap()
    hs_d = nc.dram_tensor("hs", [128, cols], f32, kind="ExternalInput").ap()
    hd_d = nc.dram_tensor("hd", [128, cols], f32, kind="ExternalInput").ap()
    w1_d = nc.dram_tensor("w1bd", [128, 128], f32, kind="ExternalInput").ap()
    w2_d = nc.dram_tensor("w2bd", [128, 128], f32, kind="ExternalInput").ap()
    w3_d = nc.dram_tensor("w3bd", [128, 128], f32, kind="ExternalInput").ap()
    fold_d = nc.dram_tensor("foldm", [128, EMBD], f32, kind="ExternalInput").ap()
    bc_d = nc.dram_tensor("bcm", [EMBD, 128], f32, kind="ExternalInput").ap()
    gb_d = nc.dram_tensor("gb", [EMBD, 2], f32, kind="ExternalInput").ap()
    out_d = nc.dram_tensor("out", [128, cols], f32, kind="ExternalOutput").ap()
    ecache_d = nc.dram_tensor("ecache", [128, cols], f32).ap()

    MM = 512  # free-dim per matmul (one PSUM bank span)
    assert tile_cols % MM == 0
    ctx = ExitStack()
    with tile.TileContext(nc) as tc, ctx:
        consts = ctx.enter_context(tc.tile_pool(name="consts", bufs=1))
        streams = ctx.enter_context(tc.tile_pool(name="streams", bufs=4))
        work = ctx.enter_context(tc.tile_pool(name="work", bufs=2))
        stats = ctx.enter_context(tc.tile_pool(name="stats", bufs=1))
        psum = ctx.enter_context(tc.tile_pool(name="psum", bufs=2, space="PSUM"))
        dram = ctx.enter_context(tc.tile_pool(name="dram", bufs=2, space="DRAM"))

        w1_sb = consts.tile([128, 128], f32, tag="w1")
        nc.sync.dma_start(w1_sb[:], w1_d[:])
        w2_sb = consts.tile([128, 128], f32, tag="w2")
        nc.sync.dma_start(w2_sb[:], w2_d[:])
        w3_sb = consts.tile([128, 128], f32, tag="w3")
        nc.sync.dma_start(w3_sb[:], w3_d[:])
        fold_sb = consts.tile([128, EMBD], f32, tag="fold")
        nc.sync.dma_start(fold_sb[:], fold_d[:])
        bc_sb = consts.tile([EMBD, 128], f32, tag="bc")
        nc.sync.dma_start(bc_sb[:], bc_d[:])
        gb_sb = consts.tile([EMBD, 2], f32, tag="gb")
        nc.sync.dma_start(gb_sb[:], gb_d[:])

        s1cols = stats.tile([128, nt], f32, tag="s1")
        s2cols = stats.tile([128, nt], f32, tag="s2")

        def load_and_matmul(t):
            ea_t = streams.tile([128, tile_cols], f32, tag="ea")
            nc.sync.dma_start(ea_t[:], ea_d[:, bass.ts(t, tile_cols)])
            hs_t = streams.tile([128, tile_cols], f32, tag="hs")
            nc.sync.dma_start(hs_t[:], hs_d[:, bass.ts(t, tile_cols)])
            hd_t = streams.tile([128, tile_cols], f32, tag="hd")
            nc.sync.dma_start(hd_t[:], hd_d[:, bass.ts(t, tile_cols)])
            pe_t = psum.tile([128, tile_cols], f32, tag="pe")
            for m in range(tile_cols // MM):
                sl = bass.ts(m, MM)
                nc.tensor.matmul(pe_t[:, sl], w1_sb[:], ea_t[:, sl],
                                 start=True, stop=False)
                nc.tensor.matmul(pe_t[:, sl], w2_sb[:], hs_t[:, sl],
                                 start=False, stop=False)
                nc.tensor.matmul(pe_t[:, sl], w3_sb[:], hd_t[:, sl],
                                 start=False, stop=True)
            return ea_t, pe_t

        # ---- pass 1: accumulate BN partial sums, cache e_nb -------------
        # Sum of squares on DVE (exact multiply) — the ACT Square table has
        # a ~0.5% systematic bias which corrupts the variance.
        for t in range(nt):
            _, pe_t = load_and_matmul(t)
            cp = work.tile([128, tile_cols], f32, tag="cp")
            nc.scalar.activation(cp[:], pe_t[:],
                                 mybir.ActivationFunctionType.Copy,
                                 accum_out=s1cols[:, t:t + 1])
            sq = work.tile([128, tile_cols], f32, tag="sq")
            nc.vector.tensor_mul(sq[:], cp[:], cp[:])
            nc.vector.tensor_reduce(s2cols[:, t:t + 1], sq[:],
                                    mybir.AxisListType.X, mybir.AluOpType.add)
            nc.sync.dma_start(ecache_d[:, bass.ts(t, tile_cols)], cp[:])

        # ---- stats: reduce, all-reduce, scale/shift ---------------------
        st_sb = stats.tile([128, 2], f32, tag="st")
        nc.vector.tensor_reduce(st_sb[:, 0:1], s1cols[:],
                                mybir.AxisListType.X, mybir.AluOpType.add)
        nc.vector.tensor_reduce(st_sb[:, 1:2], s2cols[:],
                                mybir.AxisListType.X, mybir.AluOpType.add)
        cc_in = dram.tile([128, 2], f32, tag="ccin")
        cc_out = dram.tile([128, 2], f32, tag="ccout")
        nc.sync.dma_start(cc_in[:], st_sb[:])
        nc.gpsimd.collective_compute(
            "AllReduce", mybir.AluOpType.add,
            replica_groups=[list(range(N_CORES))],
            ins=[cc_in.opt()], outs=[cc_out.opt()])
        st_ar = stats.tile([128, 2], f32, tag="star")
        nc.sync.dma_start(st_ar[:], cc_out[:])

        p16 = psum.tile([EMBD, 2], f32, tag="pe")
        nc.tensor.matmul(p16[:], fold_sb[:], st_ar[:], start=True, stop=True)
        inv_e = 1.0 / float(n_edges_total)
        m16 = stats.tile([EMBD, 1], f32, tag="m16")
        nc.scalar.activation(m16[:], p16[:, 0:1],
                             mybir.ActivationFunctionType.Copy, scale=inv_e)
        q16 = stats.tile([EMBD, 1], f32, tag="q16")
        nc.scalar.activation(q16[:], p16[:, 1:2],
                             mybir.ActivationFunctionType.Copy, scale=inv_e)
        msq = stats.tile([EMBD, 1], f32, tag="msq")
        nc.scalar.square(msq[:], m16[:])
        var = stats.tile([EMBD, 1], f32, tag="var")
        nc.vector.tensor_sub(var[:], q16[:], msq[:])
        nc.vector.tensor_scalar_add(var[:], var[:], BN_EPS)
        # Both ACT Sqrt and DVE reciprocal are approximations with ~1e-3
        # systematic bias; one Newton step each brings the error to ~1e-6.
        def recip_newton(tag, src):
            r = stats.tile([EMBD, 1], f32, tag=tag)
            nc.vector.reciprocal(r[:], src[:])
            t = stats.tile([EMBD, 1], f32, tag=tag + "t")
            nc.vector.tensor_mul(t[:], src[:], r[:])
            nc.vector.tensor_scalar(t[:], t[:], -1.0, 2.0,
                                    mybir.AluOpType.mult, mybir.AluOpType.add)
            nc.vector.tensor_mul(r[:], r[:], t[:])
            return r

        s0 = stats.tile([EMBD, 1], f32, tag="sd")
        nc.scalar.sqrt(s0[:], var[:])
        r0 = recip_newton("r0", s0)
        vr = stats.tile([EMBD, 1], f32, tag="vr")
        nc.vector.tensor_mul(vr[:], var[:], r0[:])
        s1 = stats.tile([EMBD, 1], f32, tag="s1n")
        nc.vector.tensor_add(s1[:], s0[:], vr[:])
        nc.vector.tensor_scalar_mul(s1[:], s1[:], 0.5)
        rs = recip_newton("rs", s1)
        ac16 = stats.tile([EMBD, 2], f32, tag="ac16")
        nc.vector.tensor_mul(ac16[:, 0:1], rs[:], gb_sb[:, 0:1])
        t16 = stats.tile([EMBD, 1], f32, tag="t16")
        nc.vector.tensor_mul(t16[:], m16[:], ac16[:, 0:1])
        nc.vector.tensor_sub(ac16[:, 1:2], gb_sb[:, 1:2], t16[:])
        pac = psum.tile([128, 2], f32, tag="pe")
        nc.tensor.matmul(pac[:], bc_sb[:], ac16[:], start=True, stop=True)
        ac_bc = stats.tile([128, 2], f32, tag="acbc")
        nc.scalar.copy(ac_bc[:], pac[:])

        # ---- pass 2: read cached e_nb, normalize, relu, residual --------
        for t in range(nt):
            e_t = streams.tile([128, tile_cols], f32, tag="hs")
            nc.sync.dma_start(e_t[:], ecache_d[:, bass.ts(t, tile_cols)])
            ea_t = streams.tile([128, tile_cols], f32, tag="ea")
            nc.sync.dma_start(ea_t[:], ea_d[:, bass.ts(t, tile_cols)])
            r_sb = work.tile([128, tile_cols], f32, tag="cp")
            nc.scalar.activation(r_sb[:], e_t[:],
                                 mybir.ActivationFunctionType.Relu,
                                 bias=ac_bc[:, 1:2], scale=ac_bc[:, 0:1])
            o_sb = work.tile([128, tile_cols], f32, tag="sq")
            nc.vector.tensor_add(o_sb[:], r_sb[:], ea_t[:])
            nc.sync.dma_start(out_d[:, bass.ts(t, tile_cols)], o_sb[:])

    nc.compile()
    _build_cache[key] = (nc, nt, cols)
    return _build_cache[key]


def _to_t8(arr, nt, tile_cols, ec_padded):
    """[ec_padded, 16] edge-major -> [128, nt*tile_cols] feature-major
    8-group layout: out[16g+f, t*tile_cols+j] = arr[t*G*C + g*C + j, f]."""
    a = arr.reshape(nt, GROUPS, tile_cols, EMBD)
    return np.ascontiguousarray(
        a.transpose(1, 3, 0, 2).reshape(128, nt * tile_cols))


def _from_t8(mat, nt, tile_cols, ec_padded):
    a = mat.reshape(GROUPS, EMBD, nt, tile_cols)
    return np.ascontiguousarray(
        a.transpose(2, 0, 3, 1).reshape(ec_padded, EMBD))


def kernel(x, edge_index, edge_attr, w1, b1, w2, b2, w3, b3, gamma, beta):
    x = np.asarray(x, dtype=np.float32)
    edge_index = np.asarray(edge_index)
    edge_attr = np.asarray(edge_attr, dtype=np.float32)
    w1 = np.asarray(w1, dtype=np.float32)
    w2 = np.asarray(w2, dtype=np.float32)
    w3 = np.asarray(w3, dtype=np.float32)
    gamma = np.asarray(gamma, dtype=np.float32)
    beta = np.asarray(beta, dtype=np.float32)

    n_edges = edge_attr.shape[0]
    assert n_edges % N_CORES == 0
    ec = n_edges // N_CORES
    tile_cols = 2048 if ec >= 16384 else 512
    ept = GROUPS * tile_cols
    nt = -(-ec // ept)
    ec_padded = nt * ept

    nc, nt_b, cols = _build(ec_padded, tile_cols, n_edges)

    eye = np.eye(GROUPS, dtype=np.float32)
    w1bd = np.kron(eye, w1.T.astype(np.float32)).astype(np.float32)
    w2bd = np.kron(eye, w2.T.astype(np.float32)).astype(np.float32)
    w3bd = np.kron(eye, w3.T.astype(np.float32)).astype(np.float32)
    foldm = np.tile(np.eye(EMBD, dtype=np.float32), (GROUPS, 1))
    bcm = np.tile(np.eye(EMBD, dtype=np.float32), (1, GROUPS))
    gb = np.stack([gamma, beta], axis=1).astype(np.float32)

    src = edge_index[0].astype(np.int64)
    dst = edge_index[1].astype(np.int64)

    in_maps = []
    for c in range(N_CORES):
        sl = slice(c * ec, (c + 1) * ec)
        ea_p = np.zeros((ec_padded, EMBD), np.float32)
        ea_p[:ec] = edge_attr[sl]
        hs_p = np.zeros((ec_padded, EMBD), np.float32)
        hs_p[:ec] = x[src[sl]]
        hd_p = np.zeros((ec_padded, EMBD), np.float32)
        hd_p[:ec] = x[dst[sl]]
        in_maps.append({
            "ea": _to_t8(ea_p, nt, tile_cols, ec_padded),
            "hs": _to_t8(hs_p, nt, tile_cols, ec_padded),
            "hd": _to_t8(hd_p, nt, tile_cols, ec_padded),
            "w1bd": w1bd, "w2bd": w2bd, "w3bd": w3bd,
            "foldm": foldm, "bcm": bcm, "gb": gb,
        })

    res = run_bass_kernel_spmd(nc, in_maps, list(range(N_CORES)))
    global last_exec_ns
    last_exec_ns = res.exec_time_ns

    out = np.empty((n_edges, EMBD), np.float32)
    for c in range(N_CORES):
        rows = _from_t8(np.asarray(res.results[c]["out"]), nt, tile_cols,
                        ec_padded)
        out[c * ec:(c + 1) * ec] = rows[:ec]
    return out
